# revision 20
# baseline (speedup 1.0000x reference)
"""conv_downsample_2d on 8 TRN2 cores: FIR anti-alias + 3x3 stride-2 conv.

Math: the reference is a 6x6 stride-2 conv with kernel K6 = w (*) outer(k,k)/64,
k = (1,3,3,1).  The ROW FIR factor is applied to the DATA as 3 chained 2-tap
box passes on the VectorEngine (row-shifted adds keep operands 4B-aligned ->
2x_1P mode; column-shifted adds drop to 1x, so the COLUMN factor and the 1/64
fold into the PE weights):
    K36[o,c,p,v] = sum_q w[o,c,p,q] k[v-q] / 64        (3 x 6 taps)
    y[o,i,j] = sum_{c,p,v} K36[o,c,p,v] * z[c, 2i+p, 2j+v] + b[o]
where z = 3x row-box of x zero-padded by 2.

Mapping: pure batch data-parallel, one image per NeuronCore.  Partition
= (row-band g in 0..3)*32 + channel; band g owns output rows [96g, 96(g+1))
and a block-diagonal [128,128] bf16 weight per tap computes all 4 bands in one
matmul (N=384 cols, fp32 PSUM).  N=384 single-row matmuls are the measured
optimum: each costs max(160, 173) ns (PE SBUF-latency floor) and every
multi-row out-AP pays ~3.4 ns/row extra, so blocking cannot beat 6x(1,384).

Host-side repack: x is pre-arranged per core as [128, 196, 768] f32 (4 bands
x 32 ch on partitions, 196 = 192 band rows + 4 halo rows, zero-padded), so
every strip load is ONE cast-DMA (f32->bf16 SWDGE) descriptor per partition
and needs no per-band address math or edge memsets.  y is stored bf16 as
[128, 96, 384] (band-major partitions) and re-assembled + upcast on the host
(rel-err budget 2e-2 >> bf16 noise; measured ~3.3e-3).

Pipeline (16 strips of 6 output rows per band): per strip, 12 fresh x rows
are loaded as two 6-row tiles (xa/xb ring of 3); the strip-halo rows come
from the previous xb tile (cross-tile box reads, no copy-forward).  The box
chain for strip s runs entirely on the DVE (7 ops / 42 rows ~ 17.0 us)
during strip s-1's matmul window (18.6 us); offloading chain pieces to the
GpSimd measures SLOWER despite idle capacity -- deps are encoded as
per-engine progress semaphores and the in-order DVE parks its whole queue
on the cross-engine wait (~10 us/strip).  In each body the chain is issued
BEFORE the s+2 loads: load(s+2) writes xbs[(s+2)%3] == xbs[(s-1)%3], the
halo tile chain(s) reads.  Matmuls are grouped row-major (a psum row
accumulates all 18 taps, then the Activation engine evicts it with the bias
fused); z rows are consumed incrementally, which lets strip 0 start its
first matmul ~25 us in (strip 0/1/2 use quarter- and half-strip chains with
separate small load tiles -- tile-granular DMA deps would otherwise stall
the first box pass on the whole-strip load).  One HWDGE store per strip,
split 5+1 rows on the last strip to shorten the drain.  Measured 338.8 us
vs a ~325 us floor (PE busy 300 + preamble/startup ~22 + fill gaps + tail).
"""

import numpy as np

N_BATCH = 8
C_IN = 32
C_OUT = 32
H = W = 768
HO = WO = 384
NCORES = 8

G = 4               # row bands (partition groups)
BH = HO // G        # 96 output rows per band
R = 6               # output rows per band per strip
NSTRIPS = BH // R   # 16
AR = 196            # padded x rows per band (192 + 4 halo)
ZR = 13             # z rows per strip
NTAP = 18           # 3 rows x 6 cols
ZW = 772            # z tile width: x cols -2..769

_CACHE = {}
PROFILE = False
LAST_RESULTS = None


def _host_tensors(w: np.ndarray, b: np.ndarray):
    """Block-diag per-tap weights [128,NTAP,128] bf16-layout and bias [128,1]."""
    k = np.array([1.0, 3.0, 3.0, 1.0], np.float64)
    w64 = w.astype(np.float64)
    K36 = np.zeros((C_OUT, C_IN, 3, 6), np.float64)
    for q in range(3):
        for a in range(4):
            K36[:, :, :, q + a] += w64[:, :, :, q] * (k[a] / 64.0)
    K36 = K36.astype(np.float32)
    WT = np.zeros((NTAP, 128, 128), np.float32)
    blk = K36.transpose(2, 3, 1, 0).reshape(NTAP, C_IN, C_OUT)
    for g in range(G):
        WT[:, g * 32 : g * 32 + 32, g * 32 : g * 32 + 32] = blk
    # SBUF layout [k, t, m] so the weight DMA is one contiguous descriptor
    # per partition.
    WT = np.ascontiguousarray(WT.transpose(1, 0, 2))
    BIAS = np.tile(b.astype(np.float32), G).reshape(128, 1)
    return WT, BIAS


def _pack_x(x: np.ndarray) -> np.ndarray:
    """[32,768,768] f32 -> [128,196,768]: partitions = (band g)*32 + channel,
    band g rows = padded x rows 192g-2 .. 192g+194 (2-row zero pad each end)."""
    xpad = np.zeros((C_IN, H + 4, W), np.float32)
    xpad[:, 2 : H + 2] = x
    x4 = np.stack([xpad[:, 192 * g : 192 * g + AR] for g in range(G)], axis=0)
    return np.ascontiguousarray(x4.reshape(128, AR, W))


def _build_program():
    from contextlib import ExitStack

    import concourse.bacc as bacc
    import concourse.tile as tile
    from concourse import mybir

    f32 = mybir.dt.float32
    bf16 = mybir.dt.bfloat16
    IDENT = mybir.ActivationFunctionType.Identity

    nc = bacc.Bacc(
        "TRN2", target_bir_lowering=False, debug=False, num_devices=NCORES
    )
    x_d = nc.dram_tensor("x", [128, AR, W], f32, kind="ExternalInput").ap()
    wt_d = nc.dram_tensor("wt", [128, NTAP, 128], bf16, kind="ExternalInput").ap()
    bias_d = nc.dram_tensor("bias", [128, 1], f32, kind="ExternalInput").ap()
    y_d = nc.dram_tensor("y", [128, BH, WO], bf16, kind="ExternalOutput").ap()

    with tile.TileContext(nc) as tc, ExitStack() as ctx:
        wpool = ctx.enter_context(tc.tile_pool(name="wpool", bufs=1))
        xpool = ctx.enter_context(tc.tile_pool(name="xpool", bufs=1))
        opool = ctx.enter_context(tc.tile_pool(name="opool", bufs=2))
        ppool = ctx.enter_context(tc.tile_pool(name="ppool", bufs=8, space="PSUM"))

        wt_t = wpool.tile([128, NTAP, 128], bf16)
        nc.sync.dma_start(wt_t[:], wt_d[:])
        bias_t = wpool.tile([128, 1], f32)
        nc.sync.dma_start(bias_t[:], bias_d[:])

        # x tiles: strip s fresh rows 12s+4..12s+16 as xa (6) + xb (6);
        # strip 0 rows 0..10 instead live in xq0 (6) + xq1 (4) so the first
        # box pass only waits on a 6-row load.
        xq0 = xpool.tile([128, 6, W], bf16, tag="xq0")
        xq1 = xpool.tile([128, 4, W], bf16, tag="xq1")
        xas = [xpool.tile([128, 6, W], bf16, tag=f"xa{i}", name=f"xa{i}") for i in range(3)]
        xbs = [xpool.tile([128, 6, W], bf16, tag=f"xb{i}", name=f"xb{i}") for i in range(3)]
        # box-chain intermediates (whole strip)
        d1m = xpool.tile([128, 15, W], bf16, tag="d1m")
        d2m = xpool.tile([128, 14, W], bf16, tag="d2m")
        zs = []
        for i in range(2):
            z = xpool.tile([128, ZR, ZW], bf16, tag=f"z{i}", name=f"z{i}")
            nc.gpsimd.memset(z[:, :, 0:2], 0.0)
            nc.gpsimd.memset(z[:, :, ZW - 2 : ZW], 0.0)
            zs.append(z)

        def load(t, r0, r1):
            nc.gpsimd.dma_start(t[:, 0 : r1 - r0, :], x_d[:, r0:r1, :])

        # prologue loads (descriptor-gen order = priority order)
        load(xq0, 0, 6)
        load(xq1, 6, 10)
        load(xbs[0], 10, 16)
        load(xas[1], 16, 22)
        load(xbs[1], 22, 28)

        def chain(s):
            """z[s%2] <- box^3 of x rows 12s..12s+16, entirely on the DVE.
            (Offloading part of the chain to the GpSimd looks attractive on
            paper, but the per-engine progress-semaphore dep encoding plus
            the in-order DVE parks the whole DVE queue on the cross-engine
            wait -- measured ~10 us/strip.  7 DVE ops / 42 rows ~ 17.0 us
            fits inside the 18.6 us PE window.)"""
            pxb, xa, xb = xbs[(s - 1) % 3], xas[s % 3], xbs[s % 3]
            z = zs[s % 2]
            nc.vector.tensor_add(d1m[:, 0:3, :], pxb[:, 2:5, :], pxb[:, 3:6, :])
            nc.vector.tensor_add(d1m[:, 3:4, :], pxb[:, 5:6, :], xa[:, 0:1, :])
            nc.vector.tensor_add(d1m[:, 4:9, :], xa[:, 0:5, :], xa[:, 1:6, :])
            nc.vector.tensor_add(d1m[:, 9:10, :], xa[:, 5:6, :], xb[:, 0:1, :])
            nc.vector.tensor_add(d1m[:, 10:15, :], xb[:, 0:5, :], xb[:, 1:6, :])
            nc.vector.tensor_add(d2m[:, 0:14, :], d1m[:, 0:14, :], d1m[:, 1:15, :])
            nc.vector.tensor_add(
                z[:, 0:13, 2 : W + 2], d2m[:, 0:13, :], d2m[:, 1:14, :]
            )

        def chain_h0(s):
            """First half of chain(s): z rows 0:7 only (warmup latency)."""
            pxb, xa = xbs[(s - 1) % 3], xas[s % 3]
            z = zs[s % 2]
            nc.vector.tensor_add(d1m[:, 0:3, :], pxb[:, 2:5, :], pxb[:, 3:6, :])
            nc.vector.tensor_add(d1m[:, 3:4, :], pxb[:, 5:6, :], xa[:, 0:1, :])
            nc.vector.tensor_add(d1m[:, 4:9, :], xa[:, 0:5, :], xa[:, 1:6, :])
            nc.vector.tensor_add(d2m[:, 0:8, :], d1m[:, 0:8, :], d1m[:, 1:9, :])
            nc.vector.tensor_add(
                z[:, 0:7, 2 : W + 2], d2m[:, 0:7, :], d2m[:, 1:8, :]
            )

        def chain_h1(s):
            """Second half of chain(s): z rows 7:13."""
            xa, xb = xas[s % 3], xbs[s % 3]
            z = zs[s % 2]
            nc.vector.tensor_add(d1m[:, 9:10, :], xa[:, 5:6, :], xb[:, 0:1, :])
            nc.vector.tensor_add(d1m[:, 10:15, :], xb[:, 0:5, :], xb[:, 1:6, :])
            nc.vector.tensor_add(d2m[:, 8:14, :], d1m[:, 8:14, :], d1m[:, 9:15, :])
            nc.vector.tensor_add(
                z[:, 7:13, 2 : W + 2], d2m[:, 7:13, :], d2m[:, 8:14, :]
            )

        def mm_rows(s, rows, ot):
            z = zs[s % 2]
            for r in rows:
                pt = ppool.tile([128, WO], mybir.dt.float32, tag="pt", name="pt")
                for t in range(NTAP):
                    p, v = divmod(t, 6)
                    nc.tensor.matmul(
                        pt[:], wt_t[:, t, :],
                        z[:, 2 * r + p, v : v + 2 * WO - 1 : 2],
                        start=(t == 0), stop=(t == NTAP - 1),
                    )
                nc.scalar.activation(ot[:, r, :], pt[:], IDENT, bias=bias_t[:])

        # ---- strip 0: quarter-wise chain so matmuls start on a 6-row load
        ot0 = opool.tile([128, R, WO], bf16, tag="ot")
        # q1: z rows 0..2  (x rows 0..5, all in xq0)
        nc.vector.tensor_add(d1m[:, 0:5, :], xq0[:, 0:5, :], xq0[:, 1:6, :])
        nc.vector.tensor_add(d2m[:, 0:4, :], d1m[:, 0:4, :], d1m[:, 1:5, :])
        nc.vector.tensor_add(zs[0][:, 0:3, 2 : W + 2], d2m[:, 0:3, :], d2m[:, 1:4, :])
        mm_rows(0, [0], ot0)
        # q2a: z rows 3..4  (x rows 5..7) -- lands ~2 us before q2b so the
        # row-1 matmuls restart sooner
        nc.vector.tensor_add(d1m[:, 5:6, :], xq0[:, 5:6, :], xq1[:, 0:1, :])
        nc.vector.tensor_add(d1m[:, 6:7, :], xq1[:, 0:1, :], xq1[:, 1:2, :])
        nc.vector.tensor_add(d2m[:, 4:6, :], d1m[:, 4:6, :], d1m[:, 5:7, :])
        nc.vector.tensor_add(zs[0][:, 3:5, 2 : W + 2], d2m[:, 3:5, :], d2m[:, 4:6, :])
        mm_rows(0, [1], ot0)
        # q2b: z rows 5..6  (x rows 7..9)
        nc.vector.tensor_add(d1m[:, 7:9, :], xq1[:, 1:3, :], xq1[:, 2:4, :])
        nc.vector.tensor_add(d2m[:, 6:8, :], d1m[:, 6:8, :], d1m[:, 7:9, :])
        nc.vector.tensor_add(zs[0][:, 5:7, 2 : W + 2], d2m[:, 5:7, :], d2m[:, 6:8, :])
        mm_rows(0, [2], ot0)
        # h1a: z rows 7..8  (x rows 9..11) -- row 3 restarts sooner
        nc.vector.tensor_add(d1m[:, 9:10, :], xq1[:, 3:4, :], xbs[0][:, 0:1, :])
        nc.vector.tensor_add(d1m[:, 10:11, :], xbs[0][:, 0:1, :], xbs[0][:, 1:2, :])
        nc.vector.tensor_add(d2m[:, 8:10, :], d1m[:, 8:10, :], d1m[:, 9:11, :])
        nc.vector.tensor_add(
            zs[0][:, 7:9, 2 : W + 2], d2m[:, 7:9, :], d2m[:, 8:10, :]
        )
        mm_rows(0, [3], ot0)
        # h1b: z rows 9..12  (x rows 11..15)
        nc.vector.tensor_add(d1m[:, 11:15, :], xbs[0][:, 1:5, :], xbs[0][:, 2:6, :])
        nc.vector.tensor_add(d2m[:, 10:14, :], d1m[:, 10:14, :], d1m[:, 11:15, :])
        nc.vector.tensor_add(
            zs[0][:, 9:13, 2 : W + 2], d2m[:, 9:13, :], d2m[:, 10:14, :]
        )
        mm_rows(0, [4, 5], ot0)
        load(xas[2], 28, 34)
        load(xbs[2], 34, 40)
        nc.sync.dma_start(y_d[:, 0:R, :], ot0[:])

        # ---- strips 1,2: halved chains interleaved with the matmul rows so
        # the PE restarts as soon as the first 7 z rows land (the pipeline is
        # still load-bandwidth-bound here)
        for s in (1, 2):
            # NOTE: load(s+2) targets xbs[(s+2)%3] == xbs[(s-1)%3], the halo
            # tile chain_h0(s) reads -- the chain must be issued FIRST.
            ot = opool.tile([128, R, WO], bf16, tag="ot", name="ot")
            chain_h0(s)
            mm_rows(s, [0, 1, 2], ot)
            chain_h1(s)
            load(xas[(s + 2) % 3], 12 * (s + 2) + 4, 12 * (s + 2) + 10)
            load(xbs[(s + 2) % 3], 12 * (s + 2) + 10, 12 * (s + 2) + 16)
            mm_rows(s, [3, 4, 5], ot)
            nc.sync.dma_start(y_d[:, R * s : R * (s + 1), :], ot[:])

        # ---- strips 3..15: steady state, chain one strip ahead
        for s in range(3, NSTRIPS):
            ot = opool.tile([128, R, WO], bf16, tag="ot", name="ot")
            chain_h0(s)
            chain_h1(s)
            if s + 2 < NSTRIPS:
                load(xas[(s + 2) % 3], 12 * (s + 2) + 4, 12 * (s + 2) + 10)
                load(xbs[(s + 2) % 3], 12 * (s + 2) + 10, 12 * (s + 2) + 16)
            if s < NSTRIPS - 1:
                mm_rows(s, range(R), ot)
                nc.sync.dma_start(y_d[:, R * s : R * (s + 1), :], ot[:])
            else:
                # drain: store rows 0:5 as soon as their evictions are done,
                # then the last row alone, to shorten the tail
                mm_rows(s, [0, 1, 2, 3, 4], ot)
                nc.sync.dma_start(y_d[:, R * s : R * s + 5, :], ot[:, 0:5, :])
                mm_rows(s, [5], ot)
                nc.sync.dma_start(y_d[:, R * s + 5 : R * (s + 1), :], ot[:, 5:6, :])

    nc.compile()
    return nc


def kernel(x: np.ndarray, w: np.ndarray, b: np.ndarray) -> np.ndarray:
    global LAST_RESULTS
    from concourse.bass_utils import run_bass_kernel_spmd

    x = np.asarray(x, np.float32)
    WT, BIAS = _host_tensors(np.asarray(w, np.float32), np.asarray(b, np.float32))
    import ml_dtypes

    WTb = WT.astype(ml_dtypes.bfloat16)

    if "nc" not in _CACHE:
        _CACHE["nc"] = _build_program()
    nc = _CACHE["nc"]

    in_maps = [
        {"x": _pack_x(x[n]), "wt": WTb, "bias": BIAS} for n in range(N_BATCH)
    ]
    res = run_bass_kernel_spmd(nc, in_maps, list(range(NCORES)), trace=PROFILE)
    LAST_RESULTS = res
    outs = []
    for n in range(N_BATCH):
        y4 = np.asarray(res.results[n]["y"])  # [128, 96, 384] bf16
        yn = y4.reshape(G, C_OUT, BH, WO).transpose(1, 0, 2, 3).reshape(
            C_OUT, HO, WO
        )
        outs.append(yn)
    return np.stack(outs, axis=0).astype(np.float32)
